# Initial kernel scaffold
#
"""Trainium2 Bass kernel for nn_Diag: out[n, d] = input[n, d] * W[d].

Full input [200000, 512] f32 is sharded row-wise (data parallel) across 8
NeuronCores; W [512] is replicated. Per core: [25000, 512].

Per-core layout: view each 2560-row block as [128 partitions x (20 rows * 512)]
so every DMA moves 40 KB contiguous per partition (5.24 MiB per transfer;
measured at the per-core HBM roofline, ~357 GB/s combined R+W). W is broadcast
to all 128 partitions once and replicated 20x along the free dim so each block
needs a single DVE tensor_mul. Loads and stores each alternate across the two
HWDGE rings (SyncE/ScalarE) by tile parity — measured ~5% faster than
dedicating one ring per direction; bufs=3 slots pipeline load/mul/store. The
1960 leftover rows go through a [128, 15*512] chunk plus a [40, 512] tail.
"""

import dataclasses

import numpy as np

N_CORES = 8
N_NODES = 200000
D = 512
ROWS_PER_CORE = N_NODES // N_CORES  # 25000
R = 20  # DRAM rows packed into each SBUF partition per tile
TILE_ROWS = 128 * R  # 2560
NT = ROWS_PER_CORE // TILE_ROWS  # 9 full tiles
REM = ROWS_PER_CORE - NT * TILE_ROWS  # 1960 leftover rows
BUFS = 3

_NC_CACHE = {}


def _build_nc(repeat=1):
    """Build the per-core program. `repeat` > 1 emits the full pass that many
    times back-to-back inside one NEFF (used only for wall-clock benchmarking;
    pool-slot reuse serializes iterations into one continuous tile stream)."""
    import concourse.tile as tile
    from concourse import bacc, mybir

    nc = bacc.Bacc(
        "TRN2", target_bir_lowering=False, debug=False, enable_asserts=False
    )
    f32 = mybir.dt.float32
    x = nc.dram_tensor("x", [ROWS_PER_CORE, D], f32, kind="ExternalInput").ap()
    w = nc.dram_tensor("w", [D], f32, kind="ExternalInput").ap()
    y = nc.dram_tensor("y", [ROWS_PER_CORE, D], f32, kind="ExternalOutput").ap()

    def xs(t):
        return x[t * TILE_ROWS : (t + 1) * TILE_ROWS, :].rearrange(
            "(p r) d -> p (r d)", p=128
        )

    def ys(t):
        return y[t * TILE_ROWS : (t + 1) * TILE_ROWS, :].rearrange(
            "(p r) d -> p (r d)", p=128
        )

    with tile.TileContext(nc) as tc:
        with (
            tc.tile_pool(name="wpool", bufs=1) as wpool,
            tc.tile_pool(name="data", bufs=BUFS) as data,
        ):
            wt = wpool.tile([128, D], f32)
            nc.sync.dma_start(wt[0:1, :], w[None, :])
            nc.gpsimd.partition_broadcast(wt[:], wt[0:1, :])
            # Replicate W R times along the free dim with a stride-0 read AP
            # so each big tile needs one full-width tensor_mul.
            wrep = wpool.tile([128, R * D], f32)
            src_rep = dataclasses.replace(
                wt[:, :], ap=[wt[:, :].ap[0], [0, R], wt[:, :].ap[1]]
            )
            nc.vector.tensor_copy(wrep[:].rearrange("p (r d) -> p r d", r=R), src_rep)

            for _ in range(repeat):
                for t in range(NT):
                    dtile = data.tile([128, R * D], f32, tag="dtile")
                    # alternate each direction across both HWDGE rings
                    # (measured ~5% faster than dedicated per-direction rings)
                    le = nc.sync if t % 2 == 0 else nc.scalar
                    se = nc.scalar if t % 2 == 0 else nc.sync
                    le.dma_start(dtile[:], xs(t))
                    nc.vector.tensor_mul(dtile[:], dtile[:], wrep[:])
                    se.dma_start(ys(t), dtile[:])
                # remainder: full-partition chunk (rr rows per partition) + tail
                rr = REM // 128  # 15
                base = NT * TILE_ROWS
                if rr:
                    rt0 = data.tile([128, rr * D], f32, tag="dtile", name="rembig")
                    nc.sync.dma_start(
                        rt0[:],
                        x[base : base + 128 * rr, :].rearrange(
                            "(p r) d -> p (r d)", p=128
                        ),
                    )
                    nc.vector.tensor_mul(rt0[:], rt0[:], wrep[:, : rr * D])
                    nc.scalar.dma_start(
                        y[base : base + 128 * rr, :].rearrange(
                            "(p r) d -> p (r d)", p=128
                        ),
                        rt0[:],
                    )
                tail = REM - 128 * rr  # 40
                if tail:
                    rt = data.tile([128, D], f32, tag="rem")
                    nc.sync.dma_start(rt[0:tail, :], x[base + 128 * rr :, :])
                    nc.vector.tensor_mul(rt[0:tail, :], rt[0:tail, :], wt[0:tail, :])
                    nc.scalar.dma_start(y[base + 128 * rr :, :], rt[0:tail, :])
    nc.compile()
    return nc


def _run(input, W, trace=False, repeat=1, **kw):
    """Shard, execute on 8 cores, gather. Returns (full_output, BassKernelResults)."""
    from concourse import bass_utils

    if repeat not in _NC_CACHE:
        _NC_CACHE[repeat] = _build_nc(repeat)
    nc = _NC_CACHE[repeat]

    inp = np.ascontiguousarray(np.asarray(input), dtype=np.float32)
    Wf = np.ascontiguousarray(np.asarray(W), dtype=np.float32)
    shards = np.split(inp, N_CORES, axis=0)
    in_maps = [{"x": s, "w": Wf} for s in shards]
    res = bass_utils.run_bass_kernel_spmd(
        nc, in_maps, core_ids=list(range(N_CORES)), trace=trace, **kw
    )
    out = np.concatenate([r["y"] for r in res.results], axis=0)
    return out, res


def kernel(input, A, W):
    out, _ = _run(input, W)
    return out



# revision 1
# speedup vs baseline: 1.3519x; 1.3519x over previous
"""Trainium2 Bass kernel for nn_Diag: out[n, d] = input[n, d] * W[d].

Full input [200000, 512] f32 is sharded row-wise (data parallel) across 8
NeuronCores; W [512] is replicated. Per core: [25000, 512].

Per-core layout: view each 2560-row block as [128 partitions x (20 rows * 512)]
so every DMA moves 40 KB contiguous per partition (5.24 MiB per transfer;
measured at the per-core HBM roofline, ~357 GB/s combined R+W). W is broadcast
to all 128 partitions once and replicated 20x along the free dim so each block
needs a single DVE tensor_mul. Loads and stores each alternate across the two
HWDGE rings (SyncE/ScalarE) by tile parity — measured ~5% faster than
dedicating one ring per direction; bufs=3 slots pipeline load/mul/store. The
1960 leftover rows go through a [128, 15*512] chunk plus a [40, 512] tail.
"""

import dataclasses

import numpy as np

N_CORES = 8
N_NODES = 200000
D = 512
ROWS_PER_CORE = N_NODES // N_CORES  # 25000
R = 20  # DRAM rows packed into each SBUF partition per tile
TILE_ROWS = 128 * R  # 2560
NT = ROWS_PER_CORE // TILE_ROWS  # 9 full tiles
REM = ROWS_PER_CORE - NT * TILE_ROWS  # 1960 leftover rows
BUFS = 3

_NC_CACHE = {}


def _build_nc(repeat=1):
    """Build the per-core program. `repeat` > 1 emits the full pass that many
    times back-to-back inside one NEFF (used only for wall-clock benchmarking;
    pool-slot reuse serializes iterations into one continuous tile stream)."""
    import concourse.tile as tile
    from concourse import bacc, mybir

    nc = bacc.Bacc(
        "TRN2", target_bir_lowering=False, debug=False, enable_asserts=False
    )
    f32 = mybir.dt.float32
    x = nc.dram_tensor("x", [ROWS_PER_CORE, D], f32, kind="ExternalInput").ap()
    w = nc.dram_tensor("w", [D], f32, kind="ExternalInput").ap()
    y = nc.dram_tensor("y", [ROWS_PER_CORE, D], f32, kind="ExternalOutput").ap()

    def xs(t):
        return x[t * TILE_ROWS : (t + 1) * TILE_ROWS, :].rearrange(
            "(p r) d -> p (r d)", p=128
        )

    def ys(t):
        return y[t * TILE_ROWS : (t + 1) * TILE_ROWS, :].rearrange(
            "(p r) d -> p (r d)", p=128
        )

    with tile.TileContext(nc) as tc:
        with (
            tc.tile_pool(name="wpool", bufs=1) as wpool,
            tc.tile_pool(name="data", bufs=BUFS) as data,
        ):
            wt = wpool.tile([128, D], f32)
            nc.sync.dma_start(wt[0:1, :], w[None, :])
            nc.gpsimd.partition_broadcast(wt[:], wt[0:1, :])
            # Replicate W R times along the free dim with a stride-0 read AP
            # so each big tile needs one full-width tensor_mul.
            wrep = wpool.tile([128, R * D], f32)
            src_rep = dataclasses.replace(
                wt[:, :], ap=[wt[:, :].ap[0], [0, R], wt[:, :].ap[1]]
            )
            nc.vector.tensor_copy(wrep[:].rearrange("p (r d) -> p r d", r=R), src_rep)

            for _ in range(repeat):
                for t in range(NT):
                    dtile = data.tile([128, R * D], f32, tag="dtile")
                    # alternate each direction across both HWDGE rings
                    # (measured ~5% faster than dedicated per-direction rings)
                    le = nc.sync if t % 2 == 0 else nc.scalar
                    se = nc.scalar if t % 2 == 0 else nc.sync
                    le.dma_start(dtile[:], xs(t))
                    nc.vector.tensor_mul(dtile[:], dtile[:], wrep[:])
                    se.dma_start(ys(t), dtile[:])
                # remainder: full-partition chunk (rr rows per partition) + tail
                rr = REM // 128  # 15
                base = NT * TILE_ROWS
                if rr:
                    rt0 = data.tile([128, rr * D], f32, tag="dtile", name="rembig")
                    nc.sync.dma_start(
                        rt0[:],
                        x[base : base + 128 * rr, :].rearrange(
                            "(p r) d -> p (r d)", p=128
                        ),
                    )
                    nc.vector.tensor_mul(rt0[:], rt0[:], wrep[:, : rr * D])
                    nc.scalar.dma_start(
                        y[base : base + 128 * rr, :].rearrange(
                            "(p r) d -> p (r d)", p=128
                        ),
                        rt0[:],
                    )
                tail = REM - 128 * rr  # 40
                if tail:
                    rt = data.tile([128, D], f32, tag="rem")
                    nc.sync.dma_start(rt[0:tail, :], x[base + 128 * rr :, :])
                    nc.vector.tensor_mul(rt[0:tail, :], rt[0:tail, :], wt[0:tail, :])
                    nc.scalar.dma_start(y[base + 128 * rr :, :], rt[0:tail, :])
    nc.compile()
    return nc


def _run(input, W, trace=False, repeat=1, **kw):
    """Shard, execute on 8 cores, gather. Returns (full_output, BassKernelResults)."""
    from concourse import bass_utils

    if repeat not in _NC_CACHE:
        _NC_CACHE[repeat] = _build_nc(repeat)
    nc = _NC_CACHE[repeat]

    inp = np.ascontiguousarray(np.asarray(input), dtype=np.float32)
    Wf = np.ascontiguousarray(np.asarray(W), dtype=np.float32)
    shards = np.split(inp, N_CORES, axis=0)
    in_maps = [{"x": s, "w": Wf} for s in shards]
    res = bass_utils.run_bass_kernel_spmd(
        nc, in_maps, core_ids=list(range(N_CORES)), trace=trace, **kw
    )
    out = np.concatenate([r["y"] for r in res.results], axis=0)
    return out, res


def kernel(input, A, W):
    out, _ = _run(input, W)
    return out

